# revision 22
# baseline (speedup 1.0000x reference)
"""LinearAttention Trainium2 kernel: data-parallel over batch on 8 NeuronCores.

Reference computation per batch b (C=256 channels, L=4096 seq, H=8 heads, D=64):
  qkv = w_qkv @ x[b]                    # (1536, L)
  q, k, v = split(qkv)                  # each (512, L), rows = (head, dim)
  k = softmax(k, axis=L)
  ctx[h] = k[h] @ v[h].T                # (64, 64)
  out[h] = ctx[h].T @ q[h]              # (64, L)
  y[b] = w_out @ concat(out) + b_out    # (256, L)

Key algebraic optimization: the attention output is LINEAR in q, so
  y[b] = w_out @ ctx^T @ (Wq @ x[b]) + b = (w_out @ ctx^T @ Wq) @ x[b] + b
       = MW[b] @ x[b] + b,   MW[b] a per-batch (256, 256) matrix.
This removes the Q projection GEMM and shrinks the output GEMM contraction
from 512 to 256 (PE columns per batch drop from ~161K to ~100K).

Per-core design (2 batches/core):
  - K^T, V^T computed with L on partitions (lhsT = x chunk, rhs = w^T) so the
    context matmul contracts over L on the TensorEngine. K and V share each
    x-chunk stationary back-to-back.
  - context computed TRANSPOSED per head-pair: ctxT[e,d] = sum_l v[e,l]exp(k[d,l]);
    cross-head quadrants discarded via a zeroed block-diagonal SBUF tile.
    ctx/den matmuls lag the K/V projections by three l-tiles so the PE never
    waits on the ACT engine's exp.
  - softmax denominator via tiny N=1 matmuls (lhsT = expk^T chunk, rhs = ones)
    accumulating across l-tiles -> den lands with d on partitions directly.
  - w_out folded into the context on the PE: McT[d,o] = sum_e ctxT[e,d]wo[e,o],
    scaled by 1/den[d] on the ACT copy; then MW^T[ci,co] = sum_d wq[d,ci]McT[d,co].
  - final: y = MW^T.T @ x (+ bias via a 1-row matmul, skipped when b_out == 0).
  - SOFTWARE PIPELINING: each batch's tail (1/den, McT, MW^T, final GEMM) is
    woven into the NEXT batch's lp-loop at fixed positions, so tail dependency
    chains resolve while the PE streams projections - no phase-boundary drain.
  - PSUM start=True marks the tile's whole 2KB bank "pending-zero"; only the
    chronologically first matmul per bank carries start=True, siblings' first
    writes consume the still-pending bytes.
  - all TensorE compute in bf16 (f32 PSUM accumulation).
"""

import numpy as np

B, C, L = 16, 256, 4096
HID = 512
N_CORES = 8
NB = B // N_CORES  # batches per core
CC = C // 128  # contraction chunks for the input projections (2)
LP = L // 128  # l-tiles with l on partitions (32)
LT = L // 512  # l-tiles of 512 for moving-dim matmuls (8)
PR = HID // 128  # head-pairs (4): each 128-wide chunk = 2 heads of 64
SKEW = 3  # ctx/den lag (l-tiles) behind the K/V projections

_CACHE = {}


def _build(
    reps=1,
    with_bias=True,
    kv_split=False,  # separate psk/psv pools vs one shared pool
    kv_bufs=4,  # bufs of the shared K/V psum pool (or each split pool)
    ctx_bufs=1,
    den_bufs=1,
    out_bufs=2,
    dve_scales=True,  # 1/den scale-copy on DVE (True) or ACT (False)
    skew=3,  # ctx/den lag in l-tiles
    woven=True,  # weave batch tails into the next batch's lp-loop
):
    from concourse import bacc, mybir, tile

    bf16 = mybir.dt.bfloat16
    f32 = mybir.dt.float32
    Exp = mybir.ActivationFunctionType.Exp
    Copy = mybir.ActivationFunctionType.Copy

    nc = bacc.Bacc(
        "TRN2",
        target_bir_lowering=False,
        debug=False,
        enable_asserts=False,
        num_devices=N_CORES,
    )

    x_d = nc.dram_tensor("x", [NB, CC, 128, L], bf16, kind="ExternalInput")
    wk_d = nc.dram_tensor("wk_t", [CC, 128, HID], bf16, kind="ExternalInput")
    wv_d = nc.dram_tensor("wv_t", [CC, 128, HID], bf16, kind="ExternalInput")
    wqd_d = nc.dram_tensor("wqd", [PR, 128, C], bf16, kind="ExternalInput")
    wo_d = nc.dram_tensor("wo_t", [PR, 128, C], bf16, kind="ExternalInput")
    bias_d = nc.dram_tensor("bias", [1, C], bf16, kind="ExternalInput")
    out_d = nc.dram_tensor("out", [NB, 2, 128, L], f32, kind="ExternalOutput")

    with tile.TileContext(nc) as tc:
        with (
            tc.tile_pool(name="const", bufs=1) as const,
            tc.tile_pool(name="xp", bufs=3) as xp,
            tc.tile_pool(name="big", bufs=1) as big,
            tc.tile_pool(name="small", bufs=2) as small,
            tc.tile_pool(name="ostp", bufs=3) as ostp,
            tc.tile_pool(name="ps_k", bufs=kv_bufs, space="PSUM") as ps_k,
            tc.tile_pool(name="ps_v", bufs=kv_bufs, space="PSUM") as ps_v,
            tc.tile_pool(name="ps_ctx", bufs=ctx_bufs, space="PSUM") as ps_ctx,
            tc.tile_pool(name="ps_den", bufs=den_bufs, space="PSUM") as ps_den,
            tc.tile_pool(name="ps_out", bufs=out_bufs, space="PSUM") as ps_out,
        ):
            wk = const.tile([128, CC, HID], bf16)
            wv = const.tile([128, CC, HID], bf16)
            wqd = const.tile([128, PR, C], bf16)
            wo = const.tile([128, PR, C], bf16)
            bias_sb = const.tile([1, C], bf16)
            ones_col = const.tile([128, 1], bf16)
            ones_row = const.tile([1, 512], bf16)
            ctxt_sb = const.tile([128, PR, 128], bf16)

            for cc in range(CC):
                nc.sync.dma_start(wk[:, cc, :], wk_d[cc])
                nc.sync.dma_start(wv[:, cc, :], wv_d[cc])
            for pr in range(PR):
                nc.sync.dma_start(wqd[:, pr, :], wqd_d[pr])
                nc.sync.dma_start(wo[:, pr, :], wo_d[pr])
            nc.sync.dma_start(bias_sb[:], bias_d[:])
            nc.gpsimd.memset(ones_col[:], 1.0)
            nc.gpsimd.memset(ones_row[:], 1.0)
            nc.gpsimd.memset(ctxt_sb[:], 0.0)

            def make_tail(bi, xt, ctx_ps, den_ps):
                """Tail of batch bi as a dict of lp-position -> thunks, woven
                into the NEXT batch's lp-loop (engine queues fill while the
                PE streams projections, so every dep is met on arrival)."""
                inv_den = small.tile([128, PR], f32, tag="invden")
                mct = small.tile([128, PR, C], bf16, tag="mct")
                mwt = small.tile([128, 2, C], bf16, tag="mwt")

                def at0():
                    nc.vector.reciprocal(inv_den[:], den_ps[:])
                    # block-diagonal ctxT (cross-head quadrants stay zero)
                    for pr in range(PR):
                        nc.vector.tensor_copy(
                            ctxt_sb[0:64, pr, 0:64], ctx_ps[0:64, pr, 0:64]
                        )
                        nc.vector.tensor_copy(
                            ctxt_sb[64:128, pr, 64:128], ctx_ps[64:128, pr, 64:128]
                        )

                def at3():
                    # fold w_out: McT[d, o] scaled by 1/den[d]. The scale-copy
                    # runs on DVE, not ACT: ACT is a strict FIFO and a scale
                    # wedged between exp()s would stall the psk PSUM rotation.
                    for pr in range(PR):
                        mc_ps = ps_out.tile([128, C], f32, tag="out")
                        nc.tensor.matmul(
                            mc_ps[:],
                            ctxt_sb[:, pr, :],
                            wo[:, pr, :],
                            start=True,
                            stop=True,
                        )
                        if dve_scales:
                            nc.vector.tensor_scalar_mul(
                                mct[:, pr, :], mc_ps[:], inv_den[:, pr : pr + 1]
                            )
                        else:
                            nc.scalar.activation(
                                mct[:, pr, :], mc_ps[:], Copy,
                                scale=inv_den[:, pr : pr + 1],
                            )

                def at5():
                    # fold Wq: MW^T[ci, co] = sum_d wq[d, ci] * McT[d, co]
                    mwt_ps = ps_out.tile([128, 2, C], f32, tag="out")
                    for c2 in range(2):
                        for pr in range(PR):
                            nc.tensor.matmul(
                                mwt_ps[:, c2, :],
                                wqd[:, pr, c2 * 128 : (c2 + 1) * 128],
                                mct[:, pr, :],
                                start=(c2 == 0 and pr == 0),
                                stop=(pr == PR - 1),
                                skip_group_check=True,
                            )
                    nc.vector.tensor_copy(mwt[:], mwt_ps[:])

                def psf_unit(k):
                    lt, oc2 = k // 2, k % 2

                    def run():
                        psf = ps_out.tile([128, 512], f32, tag="out")
                        for c2 in range(2):
                            nc.tensor.matmul(
                                psf[:],
                                mwt[:, c2, oc2 * 128 : (oc2 + 1) * 128],
                                xt[:, c2, lt * 512 : (lt + 1) * 512],
                                start=(c2 == 0),
                                stop=(c2 == 1 and not with_bias),
                            )
                        if with_bias:
                            nc.tensor.matmul(
                                psf[:],
                                bias_sb[0:1, oc2 * 128 : (oc2 + 1) * 128],
                                ones_row[0:1, :],
                                start=False,
                                stop=True,
                            )
                        ostg = ostp.tile([128, 512], f32, tag="ostg")
                        nc.vector.tensor_copy(ostg[:], psf[:])
                        nc.sync.dma_start(
                            out_d[bi, oc2, :, lt * 512 : (lt + 1) * 512],
                            ostg[:],
                        )

                    return run

                sched = {0: [at0], 3: [at3], 5: [at5]}
                for k in range(16):
                    sched.setdefault(8 + k, []).append(psf_unit(k))
                return sched

            pending = None  # tail of the previous batch
            for rep in range(reps):
              for bi in range(NB):
                xt = xp.tile([128, CC, L], bf16)
                for cc in range(CC):
                    nc.sync.dma_start(xt[:, cc, :], x_d[bi, cc])

                expkt = big.tile([128, LP, HID], bf16, tag="expkt")
                vt = big.tile([128, LP, HID], bf16, tag="vt")
                ctx_ps = ps_ctx.tile(
                    [128, PR, 128], f32, tag="ctx", name=f"ctx_{rep}_{bi}"
                )
                den_ps = ps_den.tile(
                    [128, PR], f32, tag="den", name=f"den_{rep}_{bi}"
                )

                def ctx_den(lp):
                    for pr in range(PR):
                        nc.tensor.matmul(
                            ctx_ps[:, pr, :],
                            vt[:, lp, pr * 128 : (pr + 1) * 128],
                            expkt[:, lp, pr * 128 : (pr + 1) * 128],
                            start=(lp == 0 and pr == 0),
                            stop=(lp == LP - 1),
                            skip_group_check=True,
                        )
                    for pr in range(PR):
                        nc.tensor.matmul(
                            den_ps[:, pr : pr + 1],
                            expkt[:, lp, pr * 128 : (pr + 1) * 128],
                            ones_col[:],
                            start=(lp == 0 and pr == 0),
                            stop=(lp == LP - 1),
                            skip_group_check=True,
                        )

                for lp in range(LP):
                    psk = ps_k.tile([128, HID], f32, tag="k")
                    psv = (ps_v if kv_split else ps_k).tile(
                        [128, HID], f32, tag="v" if kv_split else "k"
                    )
                    for cc in range(CC):
                        nc.tensor.matmul(
                            psk[:],
                            xt[:, cc, lp * 128 : (lp + 1) * 128],
                            wk[:, cc, :],
                            start=(cc == 0),
                            stop=(cc == CC - 1),
                            skip_group_check=True,
                        )
                        nc.tensor.matmul(
                            psv[:],
                            xt[:, cc, lp * 128 : (lp + 1) * 128],
                            wv[:, cc, :],
                            start=(cc == 0),
                            stop=(cc == CC - 1),
                            skip_group_check=True,
                        )
                    nc.scalar.activation(expkt[:, lp, :], psk[:], Exp)
                    nc.vector.tensor_copy(vt[:, lp, :], psv[:])
                    if pending is not None:
                        for thunk in pending.get(lp, ()):
                            thunk()
                    if lp >= skew:
                        ctx_den(lp - skew)
                for lp in range(LP - skew, LP):
                    ctx_den(lp)

                if woven:
                    pending = make_tail(bi, xt, ctx_ps, den_ps)
                else:
                    tail = make_tail(bi, xt, ctx_ps, den_ps)
                    for lp in sorted(tail):
                        for thunk in tail[lp]:
                            thunk()

            if pending is not None:
                # last batch's tail runs unwoven
                for lp in sorted(pending):
                    for thunk in pending[lp]:
                        thunk()

    nc.compile()
    return nc


def _get_nc(with_bias=True):
    key = ("nc", with_bias)
    if key not in _CACHE:
        _CACHE[key] = _build(with_bias=with_bias)
    return _CACHE[key]


def _prep_in_maps(x, w_qkv, w_out, b_out):
    import ml_dtypes

    bf16 = ml_dtypes.bfloat16
    wk_t = np.ascontiguousarray(w_qkv[512:1024].T).reshape(CC, 128, HID).astype(bf16)
    wv_t = np.ascontiguousarray(w_qkv[1024:1536].T).reshape(CC, 128, HID).astype(bf16)
    wqd = np.ascontiguousarray(w_qkv[0:512]).reshape(PR, 128, C).astype(bf16)
    wo_t = np.ascontiguousarray(w_out.T).reshape(PR, 128, C).astype(bf16)
    bias = np.ascontiguousarray(b_out.reshape(1, C)).astype(bf16)
    in_maps = []
    for c in range(N_CORES):
        xs = x[c * NB : (c + 1) * NB].reshape(NB, CC, 128, L).astype(bf16)
        in_maps.append(
            {
                "x": np.ascontiguousarray(xs),
                "wk_t": wk_t,
                "wv_t": wv_t,
                "wqd": wqd,
                "wo_t": wo_t,
                "bias": bias,
            }
        )
    return in_maps


def kernel(x, w_qkv, w_out, b_out):
    from concourse.bass_utils import run_bass_kernel_spmd

    # the bias matmuls cost ~12us/rep of PE time; skip them when b_out == 0
    with_bias = bool(np.any(np.asarray(b_out)))
    nc = _get_nc(with_bias=with_bias)
    in_maps = _prep_in_maps(
        np.asarray(x, dtype=np.float32),
        np.asarray(w_qkv, dtype=np.float32),
        np.asarray(w_out, dtype=np.float32),
        np.asarray(b_out, dtype=np.float32),
    )
    res = run_bass_kernel_spmd(nc, in_maps, core_ids=list(range(N_CORES)))
    out = np.concatenate(
        [res.results[c]["out"].reshape(NB, C, L) for c in range(N_CORES)], axis=0
    )
    return out.astype(np.float32)


# revision 33
# speedup vs baseline: 1.0921x; 1.0921x over previous
"""LinearAttention Trainium2 kernel: data-parallel over batch on 8 NeuronCores.

Reference computation per batch b (C=256 channels, L=4096 seq, H=8 heads, D=64):
  qkv = w_qkv @ x[b]                    # (1536, L)
  q, k, v = split(qkv)                  # each (512, L), rows = (head, dim)
  k = softmax(k, axis=L)
  ctx[h] = k[h] @ v[h].T                # (64, 64)
  out[h] = ctx[h].T @ q[h]              # (64, L)
  y[b] = w_out @ concat(out) + b_out    # (256, L)

Key algebraic optimization: the attention output is LINEAR in q, so
  y[b] = w_out @ ctx^T @ (Wq @ x[b]) + b = (w_out @ ctx^T @ Wq) @ x[b] + b
       = MW[b] @ x[b] + b,   MW[b] a per-batch (256, 256) matrix.
This removes the Q projection GEMM and shrinks the output GEMM contraction
from 512 to 256 (PE columns per batch drop from ~161K to ~100K).

Per-core design (2 batches/core):
  - K^T, V^T computed with L on partitions (lhsT = x chunk, rhs = w^T) so the
    context matmul contracts over L on the TensorEngine. K and V share each
    x-chunk stationary back-to-back.
  - context computed TRANSPOSED per head-pair: ctxT[e,d] = sum_l v[e,l]exp(k[d,l]);
    cross-head quadrants discarded via a zeroed block-diagonal SBUF tile.
    ctx/den matmuls lag the K/V projections by three l-tiles so the PE never
    waits on the ACT engine's exp.
  - softmax denominator via tiny N=1 matmuls (lhsT = expk^T chunk, rhs = ones)
    accumulating across l-tiles -> den lands with d on partitions directly.
  - w_out folded into the context on the PE: McT[d,o] = sum_e ctxT[e,d]wo[e,o],
    scaled by 1/den[d] on the ACT copy; then MW^T[ci,co] = sum_d wq[d,ci]McT[d,co].
  - final: y = MW^T.T @ x (+ bias via a 1-row matmul, skipped when b_out == 0).
  - SOFTWARE PIPELINING: each batch's tail (1/den, McT, MW^T, final GEMM) is
    woven into the NEXT batch's lp-loop at fixed positions, so tail dependency
    chains resolve while the PE streams projections - no phase-boundary drain.
  - PSUM start=True marks the tile's whole 2KB bank "pending-zero"; only the
    chronologically first matmul per bank carries start=True, siblings' first
    writes consume the still-pending bytes.
  - all TensorE compute in bf16 (f32 PSUM accumulation).
"""

import numpy as np

B, C, L = 16, 256, 4096
HID = 512
N_CORES = 8
NB = B // N_CORES  # batches per core
CC = C // 128  # contraction chunks for the input projections (2)
LP = L // 128  # l-tiles with l on partitions (32)
LT = L // 512  # l-tiles of 512 for moving-dim matmuls (8)
PR = HID // 128  # head-pairs (4): each 128-wide chunk = 2 heads of 64
SKEW = 3  # ctx/den lag (l-tiles) behind the K/V projections

_CACHE = {}


def _build(
    reps=1,
    with_bias=True,
    kv_split=False,  # separate psk/psv pools vs one shared pool
    kv_bufs=4,  # bufs of the shared K/V psum pool (or each split pool)
    ctx_bufs=1,
    den_bufs=1,
    out_bufs=2,
    dve_scales=True,  # 1/den scale-copy on DVE (True) or ACT (False)
    skew=3,  # ctx/den lag in l-tiles
    woven=True,  # weave batch tails into the next batch's lp-loop
    ctx_transposed=False,  # ctx with expkt stationary (den shares the LDW);
    # produces ctx as [d, e], transposed back in the tail
    out_bf16=False,  # store the output as bf16 (halves output DMA traffic)
):
    from concourse import bacc, mybir, tile

    bf16 = mybir.dt.bfloat16
    f32 = mybir.dt.float32
    Exp = mybir.ActivationFunctionType.Exp
    Copy = mybir.ActivationFunctionType.Copy

    nc = bacc.Bacc(
        "TRN2",
        target_bir_lowering=False,
        debug=False,
        enable_asserts=False,
        num_devices=N_CORES,
    )

    x_d = nc.dram_tensor("x", [NB, CC, 128, L], bf16, kind="ExternalInput")
    eye_d = nc.dram_tensor("eye", [128, 128], bf16, kind="ExternalInput")
    wk_d = nc.dram_tensor("wk_t", [CC, 128, HID], bf16, kind="ExternalInput")
    wv_d = nc.dram_tensor("wv_t", [CC, 128, HID], bf16, kind="ExternalInput")
    wqd_d = nc.dram_tensor("wqd", [PR, 128, C], bf16, kind="ExternalInput")
    wo_d = nc.dram_tensor("wo_t", [PR, 128, C], bf16, kind="ExternalInput")
    bias_d = nc.dram_tensor("bias", [1, C], bf16, kind="ExternalInput")
    out_d = nc.dram_tensor(
        "out", [NB, 2, 128, L], bf16 if out_bf16 else f32, kind="ExternalOutput"
    )

    with tile.TileContext(nc) as tc:
        with (
            tc.tile_pool(name="const", bufs=1) as const,
            tc.tile_pool(name="xp", bufs=3) as xp,
            tc.tile_pool(name="big", bufs=1) as big,
            tc.tile_pool(name="small", bufs=2) as small,
            tc.tile_pool(name="ostp", bufs=3) as ostp,
            tc.tile_pool(name="ps_k", bufs=kv_bufs, space="PSUM") as ps_k,
            tc.tile_pool(name="ps_v", bufs=kv_bufs, space="PSUM") as ps_v,
            tc.tile_pool(name="ps_ctx", bufs=ctx_bufs, space="PSUM") as ps_ctx,
            tc.tile_pool(name="ps_den", bufs=den_bufs, space="PSUM") as ps_den,
            tc.tile_pool(name="ps_out", bufs=out_bufs, space="PSUM") as ps_out,
        ):
            wk = const.tile([128, CC, HID], bf16)
            wv = const.tile([128, CC, HID], bf16)
            wqd = const.tile([128, PR, C], bf16)
            wo = const.tile([128, PR, C], bf16)
            bias_sb = const.tile([1, C], bf16)
            ones_col = const.tile([128, 1], bf16)
            ones_row = const.tile([1, 512], bf16)
            ctxt_sb = const.tile([128, PR, 128], bf16)
            eye = const.tile([128, 128], bf16)
            ctxp_sb = const.tile([128, PR, 128], bf16)
            nc.sync.dma_start(eye[:], eye_d[:])
            nc.gpsimd.memset(ctxp_sb[:], 0.0)

            for cc in range(CC):
                nc.sync.dma_start(wk[:, cc, :], wk_d[cc])
                nc.sync.dma_start(wv[:, cc, :], wv_d[cc])
            for pr in range(PR):
                nc.sync.dma_start(wqd[:, pr, :], wqd_d[pr])
                nc.sync.dma_start(wo[:, pr, :], wo_d[pr])
            nc.sync.dma_start(bias_sb[:], bias_d[:])
            nc.gpsimd.memset(ones_col[:], 1.0)
            nc.gpsimd.memset(ones_row[:], 1.0)
            nc.gpsimd.memset(ctxt_sb[:], 0.0)

            def make_tail(bi, xt, ctx_ps, den_ps):
                """Tail of batch bi as a dict of lp-position -> thunks, woven
                into the NEXT batch's lp-loop (engine queues fill while the
                PE streams projections, so every dep is met on arrival)."""
                inv_den = small.tile([128, PR], f32, tag="invden")
                mct = small.tile([128, PR, C], bf16, tag="mct")
                mwt = small.tile([128, 2, C], bf16, tag="mwt")

                def at0():
                    nc.vector.reciprocal(inv_den[:], den_ps[:])
                    # block-diagonal ctx (cross-head quadrants stay zero)
                    dst = ctxp_sb if ctx_transposed else ctxt_sb
                    for pr in range(PR):
                        nc.vector.tensor_copy(
                            dst[0:64, pr, 0:64], ctx_ps[0:64, pr, 0:64]
                        )
                        nc.vector.tensor_copy(
                            dst[64:128, pr, 64:128], ctx_ps[64:128, pr, 64:128]
                        )

                def at2():
                    # ctx arrived as [d, e]; transpose each pair back to [e, d]
                    for pr in range(PR):
                        tps = ps_out.tile([128, 128], bf16, tag="out")
                        nc.tensor.transpose(tps[:], ctxp_sb[:, pr, :], eye[:])
                        nc.vector.tensor_copy(ctxt_sb[:, pr, :], tps[:])

                def at3():
                    # fold w_out: McT[d, o] scaled by 1/den[d]. The scale-copy
                    # runs on DVE, not ACT: ACT is a strict FIFO and a scale
                    # wedged between exp()s would stall the psk PSUM rotation.
                    for pr in range(PR):
                        mc_ps = ps_out.tile([128, C], f32, tag="out")
                        nc.tensor.matmul(
                            mc_ps[:],
                            ctxt_sb[:, pr, :],
                            wo[:, pr, :],
                            start=True,
                            stop=True,
                        )
                        if dve_scales:
                            nc.vector.tensor_scalar_mul(
                                mct[:, pr, :], mc_ps[:], inv_den[:, pr : pr + 1]
                            )
                        else:
                            nc.scalar.activation(
                                mct[:, pr, :], mc_ps[:], Copy,
                                scale=inv_den[:, pr : pr + 1],
                            )

                def at5():
                    # fold Wq: MW^T[ci, co] = sum_d wq[d, ci] * McT[d, co]
                    mwt_ps = ps_out.tile([128, 2, C], f32, tag="out")
                    for c2 in range(2):
                        for pr in range(PR):
                            nc.tensor.matmul(
                                mwt_ps[:, c2, :],
                                wqd[:, pr, c2 * 128 : (c2 + 1) * 128],
                                mct[:, pr, :],
                                start=(c2 == 0 and pr == 0),
                                stop=(pr == PR - 1),
                                skip_group_check=True,
                            )
                    nc.vector.tensor_copy(mwt[:], mwt_ps[:])

                def psf_unit(k):
                    lt, oc2 = k // 2, k % 2

                    def run():
                        psf = ps_out.tile([128, 512], f32, tag="out")
                        for c2 in range(2):
                            nc.tensor.matmul(
                                psf[:],
                                mwt[:, c2, oc2 * 128 : (oc2 + 1) * 128],
                                xt[:, c2, lt * 512 : (lt + 1) * 512],
                                start=(c2 == 0),
                                stop=(c2 == 1 and not with_bias),
                            )
                        if with_bias:
                            nc.tensor.matmul(
                                psf[:],
                                bias_sb[0:1, oc2 * 128 : (oc2 + 1) * 128],
                                ones_row[0:1, :],
                                start=False,
                                stop=True,
                            )
                        ostg = ostp.tile(
                            [128, 512], bf16 if out_bf16 else f32, tag="ostg"
                        )
                        nc.vector.tensor_copy(ostg[:], psf[:])
                        nc.sync.dma_start(
                            out_d[bi, oc2, :, lt * 512 : (lt + 1) * 512],
                            ostg[:],
                        )

                    return run

                sched = {0: [at0], 3: [at3], 5: [at5]}
                if ctx_transposed:
                    sched[2] = [at2]
                for k in range(16):
                    sched.setdefault(8 + k, []).append(psf_unit(k))
                return sched

            pending = None  # tail of the previous batch
            for rep in range(reps):
              for bi in range(NB):
                xt = xp.tile([128, CC, L], bf16)
                # quarter-chunked loads (subtile deps): the first projection
                # matmuls start after ~1.5us instead of waiting for all 2MB
                for q in range(4):
                    for cc in range(CC):
                        nc.sync.dma_start(
                            xt[:, cc, q * 1024 : (q + 1) * 1024],
                            x_d[bi, cc, :, q * 1024 : (q + 1) * 1024],
                        )

                expkt = big.tile([128, LP, HID], bf16, tag="expkt")
                vt = big.tile([128, LP, HID], bf16, tag="vt")
                ctx_ps = ps_ctx.tile(
                    [128, PR, 128], f32, tag="ctx", name=f"ctx_{rep}_{bi}"
                )
                den_ps = ps_den.tile(
                    [128, PR], f32, tag="den", name=f"den_{rep}_{bi}"
                )

                def ctx_den(lp):
                    if ctx_transposed:
                        # expkt chunk is the stationary for BOTH the ctx and
                        # den matmuls -> den needs no extra weight load
                        for pr in range(PR):
                            nc.tensor.matmul(
                                ctx_ps[:, pr, :],
                                expkt[:, lp, pr * 128 : (pr + 1) * 128],
                                vt[:, lp, pr * 128 : (pr + 1) * 128],
                                start=(lp == 0 and pr == 0),
                                stop=(lp == LP - 1),
                                skip_group_check=True,
                            )
                            nc.tensor.matmul(
                                den_ps[:, pr : pr + 1],
                                expkt[:, lp, pr * 128 : (pr + 1) * 128],
                                ones_col[:],
                                start=(lp == 0 and pr == 0),
                                stop=(lp == LP - 1),
                                skip_group_check=True,
                            )
                        return
                    for pr in range(PR):
                        nc.tensor.matmul(
                            ctx_ps[:, pr, :],
                            vt[:, lp, pr * 128 : (pr + 1) * 128],
                            expkt[:, lp, pr * 128 : (pr + 1) * 128],
                            start=(lp == 0 and pr == 0),
                            stop=(lp == LP - 1),
                            skip_group_check=True,
                        )
                    for pr in range(PR):
                        nc.tensor.matmul(
                            den_ps[:, pr : pr + 1],
                            expkt[:, lp, pr * 128 : (pr + 1) * 128],
                            ones_col[:],
                            start=(lp == 0 and pr == 0),
                            stop=(lp == LP - 1),
                            skip_group_check=True,
                        )

                for lp in range(LP):
                    psk = ps_k.tile([128, HID], f32, tag="k")
                    psv = (ps_v if kv_split else ps_k).tile(
                        [128, HID], f32, tag="v" if kv_split else "k"
                    )
                    for cc in range(CC):
                        nc.tensor.matmul(
                            psk[:],
                            xt[:, cc, lp * 128 : (lp + 1) * 128],
                            wk[:, cc, :],
                            start=(cc == 0),
                            stop=(cc == CC - 1),
                            skip_group_check=True,
                        )
                        nc.tensor.matmul(
                            psv[:],
                            xt[:, cc, lp * 128 : (lp + 1) * 128],
                            wv[:, cc, :],
                            start=(cc == 0),
                            stop=(cc == CC - 1),
                            skip_group_check=True,
                        )
                    nc.scalar.activation(expkt[:, lp, :], psk[:], Exp)
                    nc.vector.tensor_copy(vt[:, lp, :], psv[:])
                    if pending is not None:
                        for thunk in pending.get(lp, ()):
                            thunk()
                    if lp >= skew:
                        ctx_den(lp - skew)
                for lp in range(LP - skew, LP):
                    ctx_den(lp)

                if woven:
                    pending = make_tail(bi, xt, ctx_ps, den_ps)
                else:
                    tail = make_tail(bi, xt, ctx_ps, den_ps)
                    for lp in sorted(tail):
                        for thunk in tail[lp]:
                            thunk()

            if pending is not None:
                # last batch's tail runs unwoven
                for lp in sorted(pending):
                    for thunk in pending[lp]:
                        thunk()

    nc.compile()
    return nc


def _get_nc(with_bias=True):
    key = ("nc", with_bias)
    if key not in _CACHE:
        _CACHE[key] = _build(with_bias=with_bias)
    return _CACHE[key]


def _prep_in_maps(x, w_qkv, w_out, b_out):
    import ml_dtypes

    bf16 = ml_dtypes.bfloat16
    wk_t = np.ascontiguousarray(w_qkv[512:1024].T).reshape(CC, 128, HID).astype(bf16)
    wv_t = np.ascontiguousarray(w_qkv[1024:1536].T).reshape(CC, 128, HID).astype(bf16)
    wqd = np.ascontiguousarray(w_qkv[0:512]).reshape(PR, 128, C).astype(bf16)
    wo_t = np.ascontiguousarray(w_out.T).reshape(PR, 128, C).astype(bf16)
    bias = np.ascontiguousarray(b_out.reshape(1, C)).astype(bf16)
    eye = np.eye(128, dtype=bf16)
    in_maps = []
    for c in range(N_CORES):
        xs = x[c * NB : (c + 1) * NB].reshape(NB, CC, 128, L).astype(bf16)
        in_maps.append(
            {
                "x": np.ascontiguousarray(xs),
                "wk_t": wk_t,
                "wv_t": wv_t,
                "wqd": wqd,
                "wo_t": wo_t,
                "bias": bias,
                "eye": eye,
            }
        )
    return in_maps


def kernel(x, w_qkv, w_out, b_out):
    from concourse.bass_utils import run_bass_kernel_spmd

    # the bias matmuls cost ~12us/rep of PE time; skip them when b_out == 0
    with_bias = bool(np.any(np.asarray(b_out)))
    nc = _get_nc(with_bias=with_bias)
    in_maps = _prep_in_maps(
        np.asarray(x, dtype=np.float32),
        np.asarray(w_qkv, dtype=np.float32),
        np.asarray(w_out, dtype=np.float32),
        np.asarray(b_out, dtype=np.float32),
    )
    res = run_bass_kernel_spmd(nc, in_maps, core_ids=list(range(N_CORES)))
    out = np.concatenate(
        [res.results[c]["out"].reshape(NB, C, L) for c in range(N_CORES)], axis=0
    )
    return out.astype(np.float32)
